# revision 1
# baseline (speedup 1.0000x reference)
"""BiMamba2Dv2 Trainium2 kernel.

8 cores = 4 batches x 2 scan directions. Each core runs a full Mamba branch
(projections + conv + selective scan) for its (batch, dir) in
feature-on-partition layout [C|Di, L]. The selective scan uses the DVE/POOL
tensor_tensor_scan primitive per (d-block, state n, quarter); per-state decay
E_n = exp(A_n * delta) comes from ScalarE with A_n baked as activation scale;
the sum over the 16 states is accumulated on TensorE via identity-matmul PSUM
accumulation. fwd+rev branch outputs are summed with a paired AllReduce; the
inter-stage LayerNorm/residual/spatial-transpose glue runs on-device, with the
rev-direction flip selected by per-core mask inputs so one SPMD program
serves all cores.
"""

import sys

for _p in ("/opt/trn_rl_repo", "/root/.axon_site/_ro/trn_rl_repo"):
    if _p not in sys.path:
        sys.path.insert(0, _p)

import numpy as np
import ml_dtypes

import concourse.bass as bass
import concourse.bacc as bacc
import concourse.tile as tile
from concourse import mybir
from concourse.bass_utils import run_bass_kernel_spmd

BF16 = ml_dtypes.bfloat16

B, H, W = 4, 48, 48
C = 192
DI = 384
NB = 3             # d-blocks of 128
NST = 16           # state dim
RNK = 12           # dt rank
L = H * W          # 2304
NQ = 4
Q = L // NQ        # 576
NCORES = 8
T_TILES = [(0, 512), (512, 512), (1024, 512), (1536, 512), (2048, 256)]
Q_TILES = [(0, 512), (512, 512), (1024, 512), (1536, 192)]  # tiles of NB*Q=1728

F32 = mybir.dt.float32
F32R = mybir.dt.float32r
BF = mybir.dt.bfloat16
MUL = mybir.AluOpType.mult
ADD = mybir.AluOpType.add
SUB = mybir.AluOpType.subtract
AFT = mybir.ActivationFunctionType


def _ap(t, free_pairs, off, parts=None):
    part_pair = t.ap[0] if parts is None else parts
    return bass.AP(tensor=t.tensor, offset=t.offset + off, ap=[part_pair] + free_pairs)


def _emit_stage(nc, pools, Wt, u_bf, sfx, A_vals, partial_dram, bc_dram, sz_dram):
    big, med, scr, ps = pools["big"], pools["med"], pools["scr"], pools["ps"]

    w_in = Wt[f"win_{sfx}"]
    w_out = Wt[f"wout_{sfx}"]
    w_xp = Wt[f"wxp_{sfx}"]
    w_dt = Wt[f"wdt_{sfx}"]
    convw = Wt[f"convw_{sfx}"]
    convb = Wt[f"convb_{sfx}"]
    dtb = Wt[f"dtb_{sfx}"]
    dvec = Wt[f"dvec_{sfx}"]
    ident = Wt["ident"]

    # ---------------- P1: in_proj / conv / x_proj / dt_proj ----------------
    xh = big.tile([128, NB * L], F32, tag="bigA", name=f"xh_{sfx}")
    for m in range(6):
        for (t0, tsz) in T_TILES:
            pt = ps.tile([128, 512], F32, tag="ps", name=f"p1_{sfx}")
            for k in range(2):
                nc.tensor.matmul(
                    pt[:, :tsz],
                    w_in[k][:, m * 128:(m + 1) * 128],
                    u_bf[k][:, t0:t0 + tsz],
                    start=(k == 0), stop=(k == 1))
            if m < 3:
                nc.vector.tensor_copy(xh[:, m * L + t0: m * L + t0 + tsz], pt[:, :tsz])
            else:
                mm = m - 3
                sg_ = scr.tile([128, 512], F32, tag="sgst", name=f"sgst_{sfx}", bufs=2)
                st_ = scr.tile([128, 512], BF, tag="szst", name=f"szst_{sfx}", bufs=2)
                nc.scalar.activation(sg_[:, :tsz], pt[:, :tsz], AFT.Sigmoid)
                nc.vector.tensor_tensor(out=st_[:, :tsz], in0=pt[:, :tsz],
                                        in1=sg_[:, :tsz], op=MUL)
                nc.gpsimd.dma_start(out=sz_dram[:, mm * L + t0: mm * L + t0 + tsz],
                                  in_=st_[:, :tsz])

    # depthwise causal conv (K=3, +bias) then silu -> xc
    cv = big.tile([128, NB * L], F32, tag="bigB", name=f"cv_{sfx}")
    xc = big.tile([128, NB * L], F32, tag="bigC", name=f"xc_{sfx}")
    for b in range(NB):
        xb = xh[:, b * L:(b + 1) * L]
        cb = cv[:, b * L:(b + 1) * L]
        nc.vector.tensor_scalar(out=cb, in0=xb, scalar1=convw[b][:, 2:3], scalar2=None, op0=MUL)
        nc.vector.scalar_tensor_tensor(
            cb[:, 1:L], xb[:, 0:L - 1], convw[b][:, 1:2], cb[:, 1:L], MUL, ADD)
        nc.vector.scalar_tensor_tensor(
            cb[:, 2:L], xb[:, 0:L - 2], convw[b][:, 0:1], cb[:, 2:L], MUL, ADD)
        xcb = xc[:, b * L:(b + 1) * L]
        nc.vector.tensor_scalar(out=cb, in0=cb, scalar1=convb[b], scalar2=None, op0=ADD)
        nc.scalar.activation(xcb, cb, AFT.Sigmoid)
        nc.vector.tensor_tensor(out=xcb, in0=cb, in1=xcb, op=MUL)

    # x_proj -> dt rows [12, L] and B/C rows [32, L] (separate m-chunks so all
    # engine APs start at partition 0)
    xdbl = med.tile([12, L], F32, tag="medA", name=f"xdbl_{sfx}")
    bcbf = med.tile([32, L], BF, tag="bcbf", name=f"bcbf_{sfx}")
    for (t0, tsz) in T_TILES:
        pt = ps.tile([12, 512], F32, tag="ps", name=f"pxp_{sfx}")
        pb = ps.tile([32, 512], F32, tag="ps", name=f"pxb_{sfx}")
        for k in range(NB):
            nc.tensor.matmul(
                pt[:, :tsz],
                w_xp[k][:, 0:RNK],
                xc[:, k * L + t0: k * L + t0 + tsz],
                start=(k == 0), stop=(k == NB - 1))
            nc.tensor.matmul(
                pb[:, :tsz],
                w_xp[k][:, RNK:44],
                xc[:, k * L + t0: k * L + t0 + tsz],
                start=(k == 0), stop=(k == NB - 1))
        nc.vector.tensor_copy(xdbl[:, t0:t0 + tsz], pt[:, :tsz])
        nc.vector.tensor_copy(bcbf[:, t0:t0 + tsz], pb[:, :tsz])

    # dt_proj + softplus -> delta
    delta = big.tile([128, NB * L], F32, tag="bigA", name=f"delta_{sfx}")
    for m in range(NB):
        for (t0, tsz) in T_TILES:
            pt = ps.tile([128, 512], F32, tag="ps", name=f"pdt_{sfx}")
            nc.tensor.matmul(
                pt[:, :tsz],
                w_dt[:, m * 128:(m + 1) * 128],
                xdbl[:, t0:t0 + tsz],
                start=True, stop=True)
            nc.scalar.activation(delta[:, m * L + t0: m * L + t0 + tsz], pt[:, :tsz],
                                 AFT.Exp, bias=dtb[m])

    for m in range(NB):
        nc.scalar.activation(delta[:, m * L:(m + 1) * L], delta[:, m * L:(m + 1) * L],
                             AFT.Ln, bias=Wt["ones_col"])

    # du = delta * xc (bf16)
    du = med.tile([128, NB * L], BF, tag="medB", name=f"du_{sfx}")
    for b in range(NB):
        nc.vector.tensor_tensor(out=du[:, b * L:(b + 1) * L],
                                in0=delta[:, b * L:(b + 1) * L],
                                in1=xc[:, b * L:(b + 1) * L], op=MUL)

    # B/C rows -> DRAM (for partition-broadcast loads)
    nc.gpsimd.dma_start(out=bc_dram[:, :], in_=bcbf)

    # ---------------- P2: selective scan (quarters) ----------------
    # DVE owns blocks 0-1; GPSIMD (POOL) owns block 2 — every
    # TensorScalarPtr op then has at most one cross-engine wait (the S3D3_TS
    # ISA struct has a single sync-wait slot).
    y = big.tile([128, NB * L], F32, tag="bigB", name=f"y_{sfx}")
    hlp_dve = None
    hlp_pool = None
    for q in range(NQ):
        qoff = q * Q
        ypA = [pools["ps_big"].tile([128, 512], F32, tag=f"ypA{b}", name=f"ypA{b}_{sfx}")
               for b in range(NB)]
        ypB = [pools["ps_big"].tile([128, 64], F32, tag=f"ypB{b}", name=f"ypB{b}_{sfx}")
               for b in range(NB)]
        hl_dve = scr.tile([128, NST * 2], BF, tag="hld", name=f"hld_{sfx}", bufs=2)
        hl_pool = scr.tile([128, NST], BF, tag="hlp", name=f"hlp_{sfx}", bufs=2)
        for n in range(NST):
            E = big.tile([128, NB * Q], F32, tag="E", name=f"E_{sfx}", bufs=2)
            nc.scalar.activation(
                _ap(E, [[Q, NB], [1, Q]], 0),
                _ap(delta, [[L, NB], [1, Q]], qoff),
                AFT.Exp, scale=float(A_vals[n]))
            bcB = scr.tile([128, Q], BF, tag="bcB", name=f"bcB_{sfx}", bufs=2)
            bcC = scr.tile([128, Q], BF, tag="bcC", name=f"bcC_{sfx}", bufs=2)
            nc.gpsimd.dma_start(
                out=bcB, in_=bc_dram.ap()[n:n + 1, qoff:qoff + Q].partition_broadcast(128))
            nc.gpsimd.dma_start(
                out=bcC, in_=bc_dram.ap()[NST + n:NST + n + 1, qoff:qoff + Q].partition_broadcast(128))
            X01 = scr.tile([128, 2 * Q], BF, tag="X01", name=f"X01_{sfx}", bufs=2)
            X2 = scr.tile([128, Q], BF, tag="X2", name=f"X2_{sfx}", bufs=2)
            nc.vector.tensor_tensor(
                out=_ap(X01, [[Q, 2], [1, Q]], 0),
                in0=_ap(du, [[L, 2], [1, Q]], qoff),
                in1=_ap(bcB, [[0, 2], [1, Q]], 0), op=MUL)
            nc.gpsimd.tensor_tensor(
                out=X2, in0=du[:, 2 * L + qoff: 2 * L + qoff + Q], in1=bcB, op=MUL)
            h01 = scr.tile([128, 2 * Q], BF, tag="h01", name=f"h01_{sfx}", bufs=1)
            h2 = scr.tile([128, Q], BF, tag="h2", name=f"h2_{sfx}", bufs=1)
            for b in range(2):
                init = 0.0 if q == 0 else hlp_dve[:, n * 2 + b: n * 2 + b + 1]
                nc.vector.tensor_tensor_scan(
                    h01[:, b * Q:(b + 1) * Q],
                    E[:, b * Q:(b + 1) * Q],
                    X01[:, b * Q:(b + 1) * Q],
                    init, MUL, ADD)
            init2 = 0.0 if q == 0 else hlp_pool[:, n: n + 1]
            nc.vector.tensor_tensor_scan(
                h2, E[:, 2 * Q:3 * Q], X2, init2, MUL, ADD)
            if q < NQ - 1:
                nc.vector.tensor_copy(
                    hl_dve[:, n * 2: n * 2 + 2],
                    _ap(h01, [[Q, 2], [1, 1]], Q - 1))
                nc.gpsimd.tensor_copy(hl_pool[:, n: n + 1], h2[:, Q - 1: Q])
            hm01 = scr.tile([128, 2 * Q], BF, tag="hm01", name=f"hm01_{sfx}", bufs=2)
            hm2 = scr.tile([128, Q], BF, tag="hm2", name=f"hm2_{sfx}", bufs=2)
            nc.vector.tensor_tensor(
                out=_ap(hm01, [[Q, 2], [1, Q]], 0),
                in0=_ap(h01, [[Q, 2], [1, Q]], 0),
                in1=_ap(bcC, [[0, 2], [1, Q]], 0), op=MUL)
            nc.gpsimd.tensor_tensor(out=hm2, in0=h2, in1=bcC, op=MUL)
            for b in range(2):
                nc.tensor.matmul(ypA[b][:, :], ident, hm01[:, b * Q: b * Q + 512],
                                 start=(n == 0), stop=(n == NST - 1))
                nc.tensor.matmul(ypB[b][:, :], ident, hm01[:, b * Q + 512: (b + 1) * Q],
                                 start=(n == 0), stop=(n == NST - 1))
            nc.tensor.matmul(ypA[2][:, :], ident, hm2[:, 0:512],
                             start=(n == 0), stop=(n == NST - 1))
            nc.tensor.matmul(ypB[2][:, :], ident, hm2[:, 512:Q],
                             start=(n == 0), stop=(n == NST - 1))
        hlp_dve = hl_dve
        hlp_pool = hl_pool
        # y = ypsum + xc * D   (per block)
        for b in range(NB):
            nc.vector.scalar_tensor_tensor(
                y[:, b * L + qoff: b * L + qoff + 512],
                xc[:, b * L + qoff: b * L + qoff + 512],
                dvec[b],
                ypA[b][:, :],
                MUL, ADD)
            nc.vector.scalar_tensor_tensor(
                y[:, b * L + qoff + 512: b * L + qoff + Q],
                xc[:, b * L + qoff + 512: b * L + qoff + Q],
                dvec[b],
                ypB[b][:, :],
                MUL, ADD)

    # ---------------- P3: gate + out_proj ----------------
    yg = med.tile([128, NB * L], BF, tag="medA", name=f"yg_{sfx}")
    for b in range(NB):
        szr = scr.tile([128, L], BF, tag="szr", name=f"szr_{sfx}", bufs=2)
        nc.gpsimd.dma_start(out=szr, in_=sz_dram[:, b * L:(b + 1) * L])
        nc.vector.tensor_tensor(out=yg[:, b * L:(b + 1) * L],
                                in0=y[:, b * L:(b + 1) * L],
                                in1=szr, op=MUL)
    for m in range(2):
        msz = 128 if m == 0 else 64
        for (t0, tsz) in T_TILES:
            pt = ps.tile([128, 512], F32, tag="ps", name=f"pout_{sfx}")
            for k in range(NB):
                nc.tensor.matmul(
                    pt[:msz, :tsz],
                    w_out[k][:, m * 128: m * 128 + msz],
                    yg[:, k * L + t0: k * L + t0 + tsz],
                    start=(k == 0), stop=(k == NB - 1))
            stg = scr.tile([128, 512], F32, tag="stg", name=f"stg_{sfx}", bufs=1)
            nc.vector.tensor_copy(stg[:msz, :tsz], pt[:msz, :tsz])
            nc.gpsimd.dma_start(out=partial_dram.ap()[m * 128: m * 128 + msz, t0:t0 + tsz],
                              in_=stg[:msz, :tsz])


def build_nc(A_vals):
    nc = bacc.Bacc("TRN2", target_bir_lowering=False, debug=False,
                   enable_asserts=False, num_devices=NCORES)

    u0_bf = nc.dram_tensor("u0_bf", [C, L], BF, kind="ExternalInput")
    xres = nc.dram_tensor("xres", [C, L], F32, kind="ExternalInput")
    mask = nc.dram_tensor("mask", [128, 1], F32, kind="ExternalInput")
    maskinv = nc.dram_tensor("maskinv", [128, 1], F32, kind="ExternalInput")
    normw = nc.dram_tensor("normw", [C, 1], F32, kind="ExternalInput")
    normb = nc.dram_tensor("normb", [C, 1], F32, kind="ExternalInput")
    ident_in = nc.dram_tensor("ident", [128, 128], BF, kind="ExternalInput")
    wdecl = {}
    for s in ("a", "b"):
        wdecl[f"win_{s}"] = nc.dram_tensor(f"win_{s}", [C, 2 * DI], BF, kind="ExternalInput")
        wdecl[f"wout_{s}"] = nc.dram_tensor(f"wout_{s}", [DI, C], BF, kind="ExternalInput")
        wdecl[f"wxp_{s}"] = nc.dram_tensor(f"wxp_{s}", [DI, 44], F32, kind="ExternalInput")
        wdecl[f"wdt_{s}"] = nc.dram_tensor(f"wdt_{s}", [RNK, DI], F32, kind="ExternalInput")
        wdecl[f"convw_{s}"] = nc.dram_tensor(f"convw_{s}", [DI, 3], F32, kind="ExternalInput")
        wdecl[f"convb_{s}"] = nc.dram_tensor(f"convb_{s}", [DI, 1], F32, kind="ExternalInput")
        wdecl[f"dtb_{s}"] = nc.dram_tensor(f"dtb_{s}", [DI, 1], F32, kind="ExternalInput")
        wdecl[f"dvec_{s}"] = nc.dram_tensor(f"dvec_{s}", [DI, 1], F32, kind="ExternalInput")
    out_full = nc.dram_tensor("out_full", [C, L], F32, kind="ExternalOutput")

    partial_a = nc.dram_tensor("partial_a", [C, L], F32)
    ssum_a = nc.dram_tensor("ssum_a", [C, L], F32)
    partial_b = nc.dram_tensor("partial_b", [C, L], F32)
    ssum_b = nc.dram_tensor("ssum_b", [C, L], F32)
    bc_dram_a = nc.dram_tensor("bc_dram_a", [32, L], BF)
    bc_dram_b = nc.dram_tensor("bc_dram_b", [32, L], BF)
    sz_dram_a = nc.dram_tensor("sz_dram_a", [128, NB * L], BF)
    sz_dram_b = nc.dram_tensor("sz_dram_b", [128, NB * L], BF)
    stats_dram = nc.dram_tensor("stats_dram", [2, L], F32)

    groups = [[b, b + 4] for b in range(B)]

    import contextlib
    with contextlib.ExitStack() as ctx:
        tc = ctx.enter_context(tile.TileContext(nc))
        pools = {
            "w": ctx.enter_context(tc.tile_pool(name="w", bufs=1)),
            "big": ctx.enter_context(tc.tile_pool(name="big", bufs=1)),
            "med": ctx.enter_context(tc.tile_pool(name="med", bufs=1)),
            "scr": ctx.enter_context(tc.tile_pool(name="scr", bufs=2)),
            "glue": ctx.enter_context(tc.tile_pool(name="glue", bufs=1)),
            "ps": ctx.enter_context(tc.tile_pool(name="ps", bufs=2, space="PSUM")),
            "ps_big": ctx.enter_context(tc.tile_pool(name="ps_big", bufs=1, space="PSUM")),
        }
        wp = pools["w"]

        Wt = {}
        for s in ("a", "b"):
            t1 = wp.tile([128, 2 * DI], BF, tag=f"win0{s}", name=f"win0{s}")
            t2 = wp.tile([64, 2 * DI], BF, tag=f"win1{s}", name=f"win1{s}")
            nc.gpsimd.dma_start(out=t1, in_=wdecl[f"win_{s}"].ap()[0:128, :])
            nc.gpsimd.dma_start(out=t2, in_=wdecl[f"win_{s}"].ap()[128:192, :])
            Wt[f"win_{s}"] = [t1, t2]
            Wt[f"wout_{s}"] = []
            for k in range(NB):
                t = wp.tile([128, C], BF, tag=f"wout{k}{s}", name=f"wout{k}{s}")
                nc.gpsimd.dma_start(out=t, in_=wdecl[f"wout_{s}"].ap()[k * 128:(k + 1) * 128, :])
                Wt[f"wout_{s}"].append(t)
            Wt[f"wxp_{s}"] = []
            for k in range(NB):
                t = wp.tile([128, 44], F32, tag=f"wxp{k}{s}", name=f"wxp{k}{s}")
                nc.gpsimd.dma_start(out=t, in_=wdecl[f"wxp_{s}"].ap()[k * 128:(k + 1) * 128, :])
                Wt[f"wxp_{s}"].append(t)
            t = wp.tile([RNK, DI], F32, tag=f"wdt{s}", name=f"wdt{s}")
            nc.gpsimd.dma_start(out=t, in_=wdecl[f"wdt_{s}"].ap()[:, :])
            Wt[f"wdt_{s}"] = t
            for nm in ("convw", "convb", "dtb", "dvec"):
                cols = 3 if nm == "convw" else 1
                lst = []
                for k in range(NB):
                    t = wp.tile([128, cols], F32, tag=f"{nm}{k}{s}", name=f"{nm}{k}{s}")
                    nc.gpsimd.dma_start(out=t, in_=wdecl[f"{nm}_{s}"].ap()[k * 128:(k + 1) * 128, :])
                    tm = wp.tile([128, cols], F32, tag=f"{nm}{k}{s}m", name=f"{nm}{k}{s}m")
                    nc.vector.tensor_copy(tm, t)
                    lst.append(tm)
                Wt[f"{nm}_{s}"] = lst
        idt = wp.tile([128, 128], BF, tag="ident", name="ident_t")
        nc.gpsimd.dma_start(out=idt, in_=ident_in.ap()[:, :])
        Wt["ident"] = idt
        nw = [wp.tile([128, 1], F32, tag="nw0", name="nw0"),
              wp.tile([64, 1], F32, tag="nw1", name="nw1")]
        nb_ = [wp.tile([128, 1], F32, tag="nb0", name="nb0"),
               wp.tile([64, 1], F32, tag="nb1", name="nb1")]
        nwd = [wp.tile([128, 1], F32, tag="nw0d", name="nw0d"),
               wp.tile([64, 1], F32, tag="nw1d", name="nw1d")]
        nbd = [wp.tile([128, 1], F32, tag="nb0d", name="nb0d"),
               wp.tile([64, 1], F32, tag="nb1d", name="nb1d")]
        nc.gpsimd.dma_start(out=nwd[0], in_=normw.ap()[0:128, :])
        nc.gpsimd.dma_start(out=nwd[1], in_=normw.ap()[128:192, :])
        nc.gpsimd.dma_start(out=nbd[0], in_=normb.ap()[0:128, :])
        nc.gpsimd.dma_start(out=nbd[1], in_=normb.ap()[128:192, :])
        for p in range(2):
            nc.vector.tensor_copy(nw[p], nwd[p])
            nc.vector.tensor_copy(nb_[p], nbd[p])
        mskd = wp.tile([128, 1], F32, tag="mskd", name="mskd")
        mskvd = wp.tile([128, 1], F32, tag="mskvd", name="mskvd")
        msk = wp.tile([128, 1], F32, tag="msk", name="msk")
        mskv = wp.tile([128, 1], F32, tag="mskv", name="mskv")
        nc.gpsimd.dma_start(out=mskd, in_=mask.ap()[:, :])
        nc.gpsimd.dma_start(out=mskvd, in_=maskinv.ap()[:, :])
        nc.vector.tensor_copy(msk, mskd)
        nc.vector.tensor_copy(mskv, mskvd)
        ones_a = wp.tile([128, 1], F32, tag="ones_a", name="ones_a")
        ones_b = wp.tile([64, 1], F32, tag="ones_b", name="ones_b")
        nc.vector.memset(ones_a, 1.0)
        nc.vector.memset(ones_b, 1.0)
        Wt["ones_col"] = ones_a

        uA = [wp.tile([128, L], BF, tag="uin0", name="uA0"),
              wp.tile([64, L], BF, tag="uin1", name="uA1")]
        nc.gpsimd.dma_start(out=uA[0], in_=u0_bf.ap()[0:128, :])
        nc.gpsimd.dma_start(out=uA[1], in_=u0_bf.ap()[128:192, :])

        _emit_stage(nc, pools, Wt, uA, "a", A_vals, partial_a, bc_dram_a, sz_dram_a)

        nc.gpsimd.collective_compute(
            "AllReduce", ADD, replica_groups=groups,
            ins=[partial_a.ap().opt()], outs=[ssum_a.ap().opt()])

        # ---------------- glue ----------------
        gl = pools["glue"]
        big = pools["big"]
        med = pools["med"]
        # packed [128, 2L]: cols 0:L = channels 0..127, cols L:2L (rows 0:64) = channels 128..191
        st = big.tile([128, 2 * L], F32, tag="bigB", name="st_g")
        fl = big.tile([128, 2 * L], F32, tag="bigC", name="fl_g")
        res = med.tile([128, 2 * L], F32, tag="medB", name="res_g")
        sq = big.tile([128, 2 * L], F32, tag="bigA", name="sq_g")
        rA = gl.tile([1, L], F32, tag="rA", name="rA_g")
        rB = gl.tile([1, L], F32, tag="rA", name="rB_g")
        epst = gl.tile([1, 1], F32, tag="epst", name="epst_g")
        ssb = med.tile([128, 2 * L], F32, tag="medA", name="ssb_g")
        nc.gpsimd.dma_start(out=ssb[:, 0:L], in_=ssum_a.ap()[0:128, :])
        nc.gpsimd.dma_start(out=ssb[0:64, L:2 * L], in_=ssum_a.ap()[128:192, :])
        for p in range(2):
            psz = 128 if p == 0 else 64
            co = p * L
            # permuted straight view & flipped view (DVE strided copies)
            nc.vector.tensor_copy(
                _ap(st, [[48, 48], [1, 48]], co, parts=[st.ap[0][0], psz]),
                _ap(ssb, [[1, 48], [48, 48]], co, parts=[ssb.ap[0][0], psz]))
            nc.gpsimd.tensor_copy(
                _ap(fl, [[48, 48], [1, 48]], co, parts=[fl.ap[0][0], psz]),
                _ap(ssb, [[-1, 48], [-48, 48]], co + L - 1, parts=[ssb.ap[0][0], psz]))
            nc.gpsimd.dma_start(out=res[0:psz, co:co + L], in_=xres.ap()[p * 128:p * 128 + psz, :])
            # select: st = st*maskinv + fl*mask
            nc.vector.tensor_scalar(out=fl[0:psz, co:co + L], in0=fl[0:psz, co:co + L],
                                    scalar1=msk[:psz, :], scalar2=None, op0=MUL)
            nc.vector.scalar_tensor_tensor(
                st[0:psz, co:co + L], st[0:psz, co:co + L], mskv[:psz, :],
                fl[0:psz, co:co + L], MUL, ADD)

        # pass 1: mean over channels via ones-matmul
        for (t0, tsz) in T_TILES:
            p1 = pools["ps"].tile([1, 512], F32, tag="ps", name="lnp1")
            for p in range(2):
                one = ones_a if p == 0 else ones_b
                nc.tensor.matmul(p1[:, :tsz], one,
                                 st[0:(128 if p == 0 else 64), p * L + t0: p * L + t0 + tsz],
                                 start=(p == 0), stop=(p == 1))
            nc.vector.tensor_copy(rA[:, t0:t0 + tsz], p1[:, :tsz])
        nc.vector.tensor_scalar(out=rA, in0=rA, scalar1=1.0 / C, scalar2=None, op0=MUL)
        nc.gpsimd.dma_start(out=stats_dram[0:1, :], in_=rA)
        mbc = big.tile([128, L], F32, tag="bigC", name="mbc_g")
        nc.gpsimd.dma_start(out=mbc, in_=stats_dram.ap()[0:1, :].partition_broadcast(128))
        # center x, square, pass 2: variance
        for p in range(2):
            psz = 128 if p == 0 else 64
            co = p * L
            nc.vector.tensor_tensor(out=st[0:psz, co:co + L], in0=st[0:psz, co:co + L],
                                    in1=mbc[0:psz, :], op=SUB)
            nc.scalar.activation(sq[0:psz, co:co + L], st[0:psz, co:co + L], AFT.Square)
        for (t0, tsz) in T_TILES:
            p2 = pools["ps"].tile([1, 512], F32, tag="ps", name="lnp2")
            for p in range(2):
                one = ones_a if p == 0 else ones_b
                nc.tensor.matmul(p2[:, :tsz], one,
                                 sq[0:(128 if p == 0 else 64), p * L + t0: p * L + t0 + tsz],
                                 start=(p == 0), stop=(p == 1))
            nc.vector.tensor_copy(rB[:, t0:t0 + tsz], p2[:, :tsz])
        nc.vector.tensor_scalar(out=rB, in0=rB, scalar1=1.0 / C, scalar2=None, op0=MUL)
        nc.vector.memset(epst, 1e-5)
        nc.scalar.activation(rB, rB, AFT.Sqrt, bias=epst)
        nc.vector.reciprocal(rB, rB)
        nc.gpsimd.dma_start(out=stats_dram[1:2, :], in_=rB)
        rbc = big.tile([128, L], F32, tag="bigA", name="rbc_g")
        nc.gpsimd.dma_start(out=rbc, in_=stats_dram.ap()[1:2, :].partition_broadcast(128))
        uB = [wp.tile([128, L], BF, tag="uin0", name="uB0"),
              wp.tile([64, L], BF, tag="uin1", name="uB1")]
        for p in range(2):
            psz = 128 if p == 0 else 64
            co = p * L
            sl = st[0:psz, co:co + L]
            nc.vector.tensor_tensor(out=sl, in0=sl, in1=rbc[0:psz, :], op=MUL)
            nc.vector.scalar_tensor_tensor(sl, sl, nw[p], res[0:psz, co:co + L], MUL, ADD)
            nc.vector.tensor_scalar(out=sl, in0=sl, scalar1=nb_[p], scalar2=None, op0=ADD)
            nc.vector.tensor_copy(uB[p], sl)

        _emit_stage(nc, pools, Wt, uB, "b", A_vals, partial_b, bc_dram_b, sz_dram_b)

        nc.gpsimd.collective_compute(
            "AllReduce", ADD, replica_groups=groups,
            ins=[partial_b.ap().opt()], outs=[ssum_b.ap().opt()])

        ob = big.tile([128, 2 * L], F32, tag="bigB", name="ob_g")
        nc.gpsimd.dma_start(out=ob[:, 0:L], in_=ssum_b.ap()[0:128, :])
        nc.gpsimd.dma_start(out=ob[0:64, L:2 * L], in_=ssum_b.ap()[128:192, :])
        nc.gpsimd.dma_start(out=out_full[0:128, :], in_=ob[:, 0:L])
        nc.gpsimd.dma_start(out=out_full[128:192, :], in_=ob[0:64, L:2 * L])

    nc.compile()
    return nc


_CACHE = {}


def make_in_maps(inputs):
    x = np.asarray(inputs["x"], np.float32)
    in_maps = []
    for core in range(NCORES):
        b, dr = core % 4, core // 4
        xw = x[b].transpose(1, 0, 2).reshape(L, C).T.copy()
        xh_ = x[b].reshape(L, C).T.copy()
        if dr == 1:
            xw = xw[:, ::-1].copy()
            xh_ = xh_[:, ::-1].copy()
        m = {
            "u0_bf": xw.astype(BF16),
            "xres": xh_.astype(np.float32),
            "mask": np.full((128, 1), float(dr), np.float32),
            "maskinv": np.full((128, 1), 1.0 - float(dr), np.float32),
            "normw": np.asarray(inputs["norm_w"], np.float32).reshape(C, 1).copy(),
            "normb": np.asarray(inputs["norm_b"], np.float32).reshape(C, 1).copy(),
            "ident": np.eye(128, dtype=BF16),
        }
        for s, i in (("a", dr), ("b", 2 + dr)):
            m[f"win_{s}"] = np.asarray(inputs["in_proj_w"][i], np.float32).T.copy().astype(BF16)
            m[f"wout_{s}"] = np.asarray(inputs["out_proj_w"][i], np.float32).T.copy().astype(BF16)
            m[f"wxp_{s}"] = np.asarray(inputs["x_proj_w"][i], np.float32).T.copy()
            m[f"wdt_{s}"] = np.asarray(inputs["dt_proj_w"][i], np.float32).T.copy()
            m[f"convw_{s}"] = np.asarray(inputs["conv_w"][i], np.float32).copy()
            m[f"convb_{s}"] = np.asarray(inputs["conv_b"][i], np.float32).reshape(DI, 1).copy()
            m[f"dtb_{s}"] = np.asarray(inputs["dt_proj_b"][i], np.float32).reshape(DI, 1).copy()
            m[f"dvec_{s}"] = np.asarray(inputs["D"][i], np.float32).reshape(DI, 1).copy()
        in_maps.append(m)
    return in_maps


def get_nc(inputs):
    if "nc" not in _CACHE:
        A_log = np.asarray(inputs["A_log"], np.float32)
        A_vals = (-np.exp(A_log[0, 0, :].astype(np.float64))).astype(np.float32)
        _CACHE["nc"] = build_nc(A_vals)
    return _CACHE["nc"]


def kernel(**inputs):
    nc = get_nc(inputs)
    in_maps = make_in_maps(inputs)
    res = run_bass_kernel_spmd(nc, in_maps, core_ids=list(range(NCORES)))
    out = np.zeros((B, H, W, C), np.float32)
    for b in range(B):
        of = res.results[b]["out_full"]
        out[b] = of.T.reshape(H, W, C)
    return out



# revision 19
# speedup vs baseline: 1.1173x; 1.1173x over previous
"""BiMamba2Dv2 Trainium2 kernel.

8 cores = 4 batches x 2 scan directions. Each core runs a full Mamba branch
(projections + conv + selective scan) for its (batch, dir) in
feature-on-partition layout [C|Di, L].

Scan phase: full-length (L=2304) tensor_tensor_scan per (state n, d-block),
split across DVE and GPSIMD. Per-state decay E_n = exp(A_n*delta) on ScalarE
(A_n baked as activation scale); X = du*B_n and hm = h*C_n as single bf16 2x
TTs against partition-broadcast B/C tiles; sum over the 16 states via
identity-matmul PSUM accumulation in 4-state groups (grouped so the B/C
broadcast tiles fit SBUF), partial-evacuated into y between groups.

The depthwise causal conv (K=3) runs on TensorE with host-prepared diagonal
stationary matrices (3 shifted accumulating matmuls per tile) and a fused
bias+SiLU on ScalarE. dt softplus and the z-gate SiLU are single fused
ScalarE activations straight out of PSUM.

fwd+rev branch outputs are summed with a paired AllReduce; the inter-stage
LayerNorm/residual/spatial-transpose glue runs on-device, with the
rev-direction flip selected by per-core mask inputs so one SPMD program
serves all cores.
"""

import sys

for _p in ("/opt/trn_rl_repo", "/root/.axon_site/_ro/trn_rl_repo"):
    if _p not in sys.path:
        sys.path.insert(0, _p)

import numpy as np
import ml_dtypes

import concourse.bass as bass
import concourse.bacc as bacc
import concourse.tile as tile
from concourse import mybir
from concourse.bass_utils import run_bass_kernel_spmd

BF16 = ml_dtypes.bfloat16

B, H, W = 4, 48, 48
C = 192
DI = 384
NB = 3             # d-blocks of 128
NST = 16           # state dim
RNK = 12           # dt rank
L = H * W          # 2304
NG = 4             # state groups (bc-broadcast granularity)
GS = NST // NG     # states per group (4)
NCORES = 8
T_TILES = [(0, 512), (512, 512), (1024, 512), (1536, 512), (2048, 256)]

F32 = mybir.dt.float32
BF = mybir.dt.bfloat16
MUL = mybir.AluOpType.mult
ADD = mybir.AluOpType.add
SUB = mybir.AluOpType.subtract
AFT = mybir.ActivationFunctionType

# scan s-index = (g*NB + b)*GS + j ; True -> DVE, False -> GPSIMD
SCAN_ON_DVE = [True for s in range(NG * NB * GS)]


def _ap(t, free_pairs, off, parts=None):
    part_pair = t.ap[0] if parts is None else parts
    return bass.AP(tensor=t.tensor, offset=t.offset + off, ap=[part_pair] + free_pairs)


def _emit_stage(nc, pools, Wt, u_bf, sfx, A_vals, partial_dram, bc_dram, sz_dram):
    big, sc, scr, ps, ps_y = (
        pools["big"], pools["sc"], pools["scr"], pools["ps"], pools["ps_y"])

    w_in = Wt[f"win_{sfx}"]
    w_out = Wt[f"wout_{sfx}"]
    w_xp = Wt[f"wxp_{sfx}"]
    w_dt = Wt[f"wdt_{sfx}"]
    convd = Wt[f"convd_{sfx}"]
    convb = Wt[f"convb_{sfx}"]
    dtb = Wt[f"dtb_{sfx}"]
    dvec = Wt[f"dvec_{sfx}"]
    ident = Wt["ident"]

    # ---------------- P1: in_proj -> xh (bf16) / silu(z) -> sz_dram ----------------
    xh = big.tile([128, NB * L], BF, tag="xh", name=f"xh_{sfx}")
    for m in range(6):
        for (t0, tsz) in T_TILES:
            pt = ps.tile([128, 512], F32, tag="ps", name=f"p1_{sfx}")
            for k in range(2):
                nc.tensor.matmul(
                    pt[:, :tsz],
                    w_in[k][:, m * 128:(m + 1) * 128],
                    u_bf[k][:, t0:t0 + tsz],
                    start=(k == 0), stop=(k == 1))
            if m < 3:
                nc.scalar.activation(xh[:, m * L + t0: m * L + t0 + tsz], pt[:, :tsz],
                                     AFT.Copy)
            else:
                mm = m - 3
                st_ = scr.tile([128, 512], BF, tag="szst", name=f"szst_{sfx}", bufs=1)
                nc.scalar.activation(st_[:, :tsz], pt[:, :tsz], AFT.Silu)
                nc.gpsimd.dma_start(out=sz_dram[:, mm * L + t0: mm * L + t0 + tsz],
                                    in_=st_[:, :tsz])

    # depthwise causal conv (K=3) on TensorE with diag stationaries; fused
    # bias+silu on ScalarE -> xc (bf16)
    xc = big.tile([128, NB * L], BF, tag="xc", name=f"xc_{sfx}")
    for b in range(NB):
        for (t0, tsz) in T_TILES:
            pc = ps.tile([128, 512], F32, tag="ps", name=f"pcv_{sfx}")
            # y(t) = w2*x(t) + w1*x(t-1) + w0*x(t-2), causal
            nc.tensor.matmul(pc[:, :tsz], convd[b][2],
                             xh[:, b * L + t0: b * L + t0 + tsz],
                             start=True, stop=False)
            for k, sh in ((1, 1), (0, 2)):
                if t0 >= sh:
                    nc.tensor.matmul(pc[:, :tsz], convd[b][k],
                                     xh[:, b * L + t0 - sh: b * L + t0 - sh + tsz],
                                     start=False, stop=(k == 0))
                else:
                    nc.tensor.matmul(pc[:, sh:tsz], convd[b][k],
                                     xh[:, b * L: b * L + tsz - sh],
                                     start=False, stop=(k == 0))
            nc.scalar.activation(xc[:, b * L + t0: b * L + t0 + tsz], pc[:, :tsz],
                                 AFT.Silu, bias=convb[b])

    # x_proj -> dt rows [12, L] (bf16) and B/C rows [32, L] (bf16)
    xdbl = sc.tile([12, L], BF, tag="X", name=f"xdbl_{sfx}", bufs=2)
    bcbf = sc.tile([32, L], BF, tag="h", name=f"bcbf_{sfx}", bufs=2)
    for (t0, tsz) in T_TILES:
        pt = ps.tile([12, 512], F32, tag="ps", name=f"pxp_{sfx}")
        pb = ps.tile([32, 512], F32, tag="ps", name=f"pxb_{sfx}")
        for k in range(NB):
            nc.tensor.matmul(
                pt[:, :tsz],
                w_xp[k][:, 0:RNK],
                xc[:, k * L + t0: k * L + t0 + tsz],
                start=(k == 0), stop=(k == NB - 1))
            nc.tensor.matmul(
                pb[:, :tsz],
                w_xp[k][:, RNK:44],
                xc[:, k * L + t0: k * L + t0 + tsz],
                start=(k == 0), stop=(k == NB - 1))
        nc.scalar.activation(xdbl[:, t0:t0 + tsz], pt[:, :tsz], AFT.Copy)
        nc.scalar.activation(bcbf[:, t0:t0 + tsz], pb[:, :tsz], AFT.Copy)

    # B/C rows -> DRAM (for partition-broadcast loads)
    nc.gpsimd.dma_start(out=bc_dram[:, :], in_=bcbf)

    # dt_proj + softplus (as exp then ln(1+x)) -> delta (bf16)
    delta = big.tile([128, NB * L], BF, tag="delta", name=f"delta_{sfx}")
    for m in range(NB):
        for (t0, tsz) in T_TILES:
            pt = ps.tile([128, 512], F32, tag="ps", name=f"pdt_{sfx}")
            nc.tensor.matmul(
                pt[:, :tsz],
                w_dt[:, m * 128:(m + 1) * 128],
                xdbl[:, t0:t0 + tsz],
                start=True, stop=True)
            nc.scalar.activation(delta[:, m * L + t0: m * L + t0 + tsz], pt[:, :tsz],
                                 AFT.Exp, bias=dtb[m])
    for m in range(NB):
        nc.scalar.activation(delta[:, m * L:(m + 1) * L], delta[:, m * L:(m + 1) * L],
                             AFT.Ln, bias=1.0)

    # du = delta * xc (bf16 2x)
    du = big.tile([128, NB * L], BF, tag="du", name=f"du_{sfx}")
    for b in range(NB):
        nc.gpsimd.tensor_tensor(out=du[:, b * L:(b + 1) * L],
                                in0=delta[:, b * L:(b + 1) * L],
                                in1=xc[:, b * L:(b + 1) * L], op=MUL)

    # ---------------- P2: selective scan (full-L, state groups) ----------------
    y = big.tile([128, NB * L], BF, tag="y", name=f"y_{sfx}")
    bcB_tiles = [None] * NG
    bcC_tiles = [None] * NG

    def issue_bc(g):
        bcB = pools["bc"].tile([128, GS * L], BF, tag="bcB", name=f"bcB_{sfx}", bufs=2)
        bcC = pools["bc"].tile([128, GS * L], BF, tag="bcC", name=f"bcC_{sfx}", bufs=2)
        for j in range(GS):
            n = GS * g + j
            nc.gpsimd.dma_start(
                out=bcB[:, j * L:(j + 1) * L],
                in_=bc_dram.ap()[n:n + 1, :].partition_broadcast(128))
            nc.gpsimd.dma_start(
                out=bcC[:, j * L:(j + 1) * L],
                in_=bc_dram.ap()[NST + n:NST + n + 1, :].partition_broadcast(128))
        bcB_tiles[g] = bcB
        bcC_tiles[g] = bcC

    issue_bc(0)
    for g in range(NG):
        if g + 1 < NG:
            issue_bc(g + 1)  # prefetch next group's broadcasts
        bcB, bcC = bcB_tiles[g], bcC_tiles[g]
        for b in range(NB):
            yps = ps_y.tile([128, L], F32, tag="ypy", name=f"ypy_{sfx}")
            for j in range(GS):
                n = GS * g + j
                s = (g * NB + b) * GS + j
                if s % 2 == 0:
                    E = big.tile([128, L], F32, tag="xh", name=f"E_{sfx}")
                else:
                    E = sc.tile([128, L], F32, tag="E1", name=f"E1_{sfx}")
                nc.scalar.activation(E, delta[:, b * L:(b + 1) * L],
                                     AFT.Exp, scale=float(A_vals[n]))
                X = sc.tile([128, L], BF, tag="X", name=f"Xs_{sfx}", bufs=2)
                nc.vector.tensor_tensor(out=X, in0=du[:, b * L:(b + 1) * L],
                                        in1=bcB[:, j * L:(j + 1) * L], op=MUL)
                h = sc.tile([128, L], BF, tag="h", name=f"hs_{sfx}", bufs=2)
                eng = nc.vector if SCAN_ON_DVE[s] else nc.gpsimd
                eng.tensor_tensor_scan(h, E, X, 0.0, MUL, ADD)
                hm = Wt["wp"].tile([128, L], BF, tag=("uin0" if j % 2 == 0 else "uin1"),
                                   name=f"hm_{sfx}")
                nc.vector.tensor_tensor(out=hm, in0=h,
                                        in1=bcC[:, j * L:(j + 1) * L], op=MUL)
                for (t0, tsz) in T_TILES:
                    nc.tensor.matmul(yps[:, t0:t0 + tsz], ident, hm[:, t0:t0 + tsz],
                                     start=(j == 0), stop=(j == GS - 1))
            # partial-evacuate this group's state-sum into y
            if g == 0:
                nc.vector.scalar_tensor_tensor(
                    y[:, b * L:(b + 1) * L],
                    xc[:, b * L:(b + 1) * L],
                    dvec[b],
                    yps[:, :],
                    MUL, ADD)
            else:
                nc.vector.tensor_tensor(
                    out=y[:, b * L:(b + 1) * L],
                    in0=y[:, b * L:(b + 1) * L],
                    in1=yps[:, :], op=ADD)

    # ---------------- P3: gate + out_proj ----------------
    for (t0, tsz) in T_TILES:
        ygc = []
        for k in range(NB):
            szr = scr.tile([128, 512], BF, tag="szr", name=f"szr_{sfx}", bufs=1)
            nc.scalar.dma_start(out=szr[:, :tsz], in_=sz_dram[:, k * L + t0: k * L + t0 + tsz])
            yg = scr.tile([128, 512], BF, tag=f"ygc{k}", name=f"ygc{k}_{sfx}", bufs=1)
            nc.gpsimd.tensor_tensor(out=yg[:, :tsz], in0=y[:, k * L + t0: k * L + t0 + tsz],
                                    in1=szr[:, :tsz], op=MUL)
            ygc.append(yg)
        for m in range(2):
            msz = 128 if m == 0 else 64
            pt = ps.tile([128, 512], F32, tag="ps", name=f"pout_{sfx}")
            for k in range(NB):
                nc.tensor.matmul(
                    pt[:msz, :tsz],
                    w_out[k][:, m * 128: m * 128 + msz],
                    ygc[k][:, :tsz],
                    start=(k == 0), stop=(k == NB - 1))
            stg = scr.tile([128, 512], F32, tag="szst", name=f"stg_{sfx}", bufs=1)
            nc.scalar.activation(stg[:msz, :tsz], pt[:msz, :tsz], AFT.Copy)
            nc.gpsimd.dma_start(out=partial_dram.ap()[m * 128: m * 128 + msz, t0:t0 + tsz],
                                in_=stg[:msz, :tsz])


def build_nc(A_vals):
    nc = bacc.Bacc("TRN2", target_bir_lowering=False, debug=False,
                   enable_asserts=False, num_devices=NCORES)

    u0_bf = nc.dram_tensor("u0_bf", [C, L], BF, kind="ExternalInput")
    xres = nc.dram_tensor("xres", [C, L], F32, kind="ExternalInput")
    mask = nc.dram_tensor("mask", [128, 1], F32, kind="ExternalInput")
    maskinv = nc.dram_tensor("maskinv", [128, 1], F32, kind="ExternalInput")
    normw = nc.dram_tensor("normw", [C, 1], F32, kind="ExternalInput")
    normb = nc.dram_tensor("normb", [C, 1], F32, kind="ExternalInput")
    ident_in = nc.dram_tensor("ident", [128, 128], BF, kind="ExternalInput")
    wdecl = {}
    for s in ("a", "b"):
        wdecl[f"win_{s}"] = nc.dram_tensor(f"win_{s}", [C, 2 * DI], BF, kind="ExternalInput")
        wdecl[f"wout_{s}"] = nc.dram_tensor(f"wout_{s}", [DI, C], BF, kind="ExternalInput")
        wdecl[f"wxp_{s}"] = nc.dram_tensor(f"wxp_{s}", [DI, 44], BF, kind="ExternalInput")
        wdecl[f"wdt_{s}"] = nc.dram_tensor(f"wdt_{s}", [RNK, DI], BF, kind="ExternalInput")
        wdecl[f"convd_{s}"] = nc.dram_tensor(f"convd_{s}", [9 * 128, 128], BF, kind="ExternalInput")
        wdecl[f"convb_{s}"] = nc.dram_tensor(f"convb_{s}", [DI, 1], F32, kind="ExternalInput")
        wdecl[f"dtb_{s}"] = nc.dram_tensor(f"dtb_{s}", [DI, 1], F32, kind="ExternalInput")
        wdecl[f"dvec_{s}"] = nc.dram_tensor(f"dvec_{s}", [DI, 1], F32, kind="ExternalInput")
    out_full = nc.dram_tensor("out_full", [C, L], F32, kind="ExternalOutput")

    partial_a = nc.dram_tensor("partial_a", [C, L], F32)
    ssum_a = nc.dram_tensor("ssum_a", [C, L], F32)
    partial_b = nc.dram_tensor("partial_b", [C, L], F32)
    ssum_b = nc.dram_tensor("ssum_b", [C, L], F32)
    bc_dram_a = nc.dram_tensor("bc_dram_a", [32, L], BF)
    bc_dram_b = nc.dram_tensor("bc_dram_b", [32, L], BF)
    sz_dram_a = nc.dram_tensor("sz_dram_a", [128, NB * L], BF)
    sz_dram_b = nc.dram_tensor("sz_dram_b", [128, NB * L], BF)
    stats_dram = nc.dram_tensor("stats_dram", [2, L], F32)

    groups = [[b, b + 4] for b in range(B)]

    import contextlib
    with contextlib.ExitStack() as ctx:
        tc = ctx.enter_context(tile.TileContext(nc))
        pools = {
            "w": ctx.enter_context(tc.tile_pool(name="w", bufs=1)),
            "big": ctx.enter_context(tc.tile_pool(name="big", bufs=1)),
            "bc": ctx.enter_context(tc.tile_pool(name="bc", bufs=1)),
            "sc": ctx.enter_context(tc.tile_pool(name="sc", bufs=1)),
            "scr": ctx.enter_context(tc.tile_pool(name="scr", bufs=2)),
            "ps": ctx.enter_context(tc.tile_pool(name="ps", bufs=2, space="PSUM")),
            "ps_y": ctx.enter_context(tc.tile_pool(name="ps_y", bufs=1, space="PSUM")),
        }
        wp = pools["w"]

        Wt = {"wp": wp}
        for s in ("a", "b"):
            t1 = wp.tile([128, 2 * DI], BF, tag=f"win0{s}", name=f"win0{s}")
            t2 = wp.tile([64, 2 * DI], BF, tag=f"win1{s}", name=f"win1{s}")
            nc.gpsimd.dma_start(out=t1, in_=wdecl[f"win_{s}"].ap()[0:128, :])
            nc.gpsimd.dma_start(out=t2, in_=wdecl[f"win_{s}"].ap()[128:192, :])
            Wt[f"win_{s}"] = [t1, t2]
            Wt[f"wout_{s}"] = []
            for k in range(NB):
                t = wp.tile([128, C], BF, tag=f"wout{k}{s}", name=f"wout{k}{s}")
                nc.gpsimd.dma_start(out=t, in_=wdecl[f"wout_{s}"].ap()[k * 128:(k + 1) * 128, :])
                Wt[f"wout_{s}"].append(t)
            Wt[f"wxp_{s}"] = []
            for k in range(NB):
                t = wp.tile([128, 44], BF, tag=f"wxp{k}{s}", name=f"wxp{k}{s}")
                nc.gpsimd.dma_start(out=t, in_=wdecl[f"wxp_{s}"].ap()[k * 128:(k + 1) * 128, :])
                Wt[f"wxp_{s}"].append(t)
            t = wp.tile([RNK, DI], BF, tag=f"wdt{s}", name=f"wdt{s}")
            nc.gpsimd.dma_start(out=t, in_=wdecl[f"wdt_{s}"].ap()[:, :])
            Wt[f"wdt_{s}"] = t
            # conv diag stationaries [block][k] = [128,128] bf16
            Wt[f"convd_{s}"] = []
            for b in range(NB):
                ks = []
                for k in range(3):
                    t = wp.tile([128, 128], BF, tag=f"cvd{b}{k}{s}", name=f"cvd{b}{k}{s}")
                    r0 = (b * 3 + k) * 128
                    nc.gpsimd.dma_start(out=t, in_=wdecl[f"convd_{s}"].ap()[r0:r0 + 128, :])
                    ks.append(t)
                Wt[f"convd_{s}"].append(ks)
            for nm in ("convb", "dtb", "dvec"):
                lst = []
                for k in range(NB):
                    t = wp.tile([128, 1], F32, tag=f"{nm}{k}{s}", name=f"{nm}{k}{s}")
                    nc.gpsimd.dma_start(out=t, in_=wdecl[f"{nm}_{s}"].ap()[k * 128:(k + 1) * 128, :])
                    tm = wp.tile([128, 1], F32, tag=f"{nm}{k}{s}m", name=f"{nm}{k}{s}m")
                    nc.vector.tensor_copy(tm, t)
                    lst.append(tm)
                Wt[f"{nm}_{s}"] = lst
        idt = wp.tile([128, 128], BF, tag="ident", name="ident_t")
        nc.gpsimd.dma_start(out=idt, in_=ident_in.ap()[:, :])
        Wt["ident"] = idt
        nw = [wp.tile([128, 1], F32, tag="nw0", name="nw0"),
              wp.tile([64, 1], F32, tag="nw1", name="nw1")]
        nb_ = [wp.tile([128, 1], F32, tag="nb0", name="nb0"),
               wp.tile([64, 1], F32, tag="nb1", name="nb1")]
        nwd = [wp.tile([128, 1], F32, tag="nw0d", name="nw0d"),
               wp.tile([64, 1], F32, tag="nw1d", name="nw1d")]
        nbd = [wp.tile([128, 1], F32, tag="nb0d", name="nb0d"),
               wp.tile([64, 1], F32, tag="nb1d", name="nb1d")]
        nc.gpsimd.dma_start(out=nwd[0], in_=normw.ap()[0:128, :])
        nc.gpsimd.dma_start(out=nwd[1], in_=normw.ap()[128:192, :])
        nc.gpsimd.dma_start(out=nbd[0], in_=normb.ap()[0:128, :])
        nc.gpsimd.dma_start(out=nbd[1], in_=normb.ap()[128:192, :])
        for p in range(2):
            nc.vector.tensor_copy(nw[p], nwd[p])
            nc.vector.tensor_copy(nb_[p], nbd[p])
        mskd = wp.tile([128, 1], F32, tag="mskd", name="mskd")
        mskvd = wp.tile([128, 1], F32, tag="mskvd", name="mskvd")
        msk = wp.tile([128, 1], F32, tag="msk", name="msk")
        mskv = wp.tile([128, 1], F32, tag="mskv", name="mskv")
        nc.gpsimd.dma_start(out=mskd, in_=mask.ap()[:, :])
        nc.gpsimd.dma_start(out=mskvd, in_=maskinv.ap()[:, :])
        nc.vector.tensor_copy(msk, mskd)
        nc.vector.tensor_copy(mskv, mskvd)
        ones_a = wp.tile([128, 1], F32, tag="ones_a", name="ones_a")
        ones_b = wp.tile([64, 1], F32, tag="ones_b", name="ones_b")
        nc.vector.memset(ones_a, 1.0)
        nc.vector.memset(ones_b, 1.0)

        uA = [wp.tile([128, L], BF, tag="uin0", name="uA0"),
              wp.tile([64, L], BF, tag="uin1", name="uA1")]
        nc.gpsimd.dma_start(out=uA[0], in_=u0_bf.ap()[0:128, :])
        nc.gpsimd.dma_start(out=uA[1], in_=u0_bf.ap()[128:192, :])

        _emit_stage(nc, pools, Wt, uA, "a", A_vals, partial_a, bc_dram_a, sz_dram_a)

        nc.gpsimd.collective_compute(
            "AllReduce", ADD, replica_groups=groups,
            ins=[partial_a.ap().opt()], outs=[ssum_a.ap().opt()])

        # ---------------- glue ----------------
        big = pools["big"]
        bcp = pools["bc"]
        sc = pools["sc"]
        # packed [128, 2L]: cols 0:L = channels 0..127, cols L:2L (rows 0:64) = channels 128..191
        st = bcp.tile([128, 2 * L], F32, tag="bcB", name="st_g", bufs=2)
        fl = bcp.tile([128, 2 * L], F32, tag="bcB", name="fl_g", bufs=2)
        res = bcp.tile([128, 2 * L], F32, tag="bcC", name="res_g", bufs=2)
        sq = bcp.tile([128, 2 * L], F32, tag="bcC", name="sq_g", bufs=2)
        rA = sc.tile([1, L], F32, tag="E1", name="rA_g")
        rB = sc.tile([1, L], F32, tag="X", name="rB_g", bufs=2)
        epst = wp.tile([1, 1], F32, tag="epst", name="epst_g")
        for p in range(2):
            psz = 128 if p == 0 else 64
            co = p * L
            # stage linear ssum into fl, permute to (w h)->(h w) order in st,
            # then overwrite fl with the 1-D reversal of st (= the rev view)
            nc.gpsimd.dma_start(out=fl[0:psz, co:co + L],
                                in_=ssum_a.ap()[p * 128:p * 128 + psz, :])
            nc.vector.tensor_copy(
                _ap(st, [[48, 48], [1, 48]], co, parts=[st.ap[0][0], psz]),
                _ap(fl, [[1, 48], [48, 48]], co, parts=[fl.ap[0][0], psz]))
            nc.gpsimd.tensor_copy(
                _ap(fl, [[1, L]], co, parts=[fl.ap[0][0], psz]),
                _ap(st, [[-1, L]], co + L - 1, parts=[st.ap[0][0], psz]))
            nc.gpsimd.dma_start(out=res[0:psz, co:co + L], in_=xres.ap()[p * 128:p * 128 + psz, :])
            # select: st = st*maskinv + fl*mask
            nc.vector.tensor_scalar(out=fl[0:psz, co:co + L], in0=fl[0:psz, co:co + L],
                                    scalar1=msk[:psz, :], scalar2=None, op0=MUL)
            nc.vector.scalar_tensor_tensor(
                st[0:psz, co:co + L], st[0:psz, co:co + L], mskv[:psz, :],
                fl[0:psz, co:co + L], MUL, ADD)

        # pass 1: mean over channels via ones-matmul
        for (t0, tsz) in T_TILES:
            p1 = pools["ps"].tile([1, 512], F32, tag="ps", name="lnp1")
            for p in range(2):
                one = ones_a if p == 0 else ones_b
                nc.tensor.matmul(p1[:, :tsz], one,
                                 st[0:(128 if p == 0 else 64), p * L + t0: p * L + t0 + tsz],
                                 start=(p == 0), stop=(p == 1))
            nc.vector.tensor_copy(rA[:, t0:t0 + tsz], p1[:, :tsz])
        nc.vector.tensor_scalar(out=rA, in0=rA, scalar1=1.0 / C, scalar2=None, op0=MUL)
        nc.gpsimd.dma_start(out=stats_dram[0:1, :], in_=rA)
        mbc = big.tile([128, L], F32, tag="y", name="mbc_g")
        nc.gpsimd.dma_start(out=mbc, in_=stats_dram.ap()[0:1, :].partition_broadcast(128))
        # center x, square, pass 2: variance
        for p in range(2):
            psz = 128 if p == 0 else 64
            co = p * L
            nc.vector.tensor_tensor(out=st[0:psz, co:co + L], in0=st[0:psz, co:co + L],
                                    in1=mbc[0:psz, :], op=SUB)
            nc.scalar.activation(sq[0:psz, co:co + L], st[0:psz, co:co + L], AFT.Square)
        for (t0, tsz) in T_TILES:
            p2 = pools["ps"].tile([1, 512], F32, tag="ps", name="lnp2")
            for p in range(2):
                one = ones_a if p == 0 else ones_b
                nc.tensor.matmul(p2[:, :tsz], one,
                                 sq[0:(128 if p == 0 else 64), p * L + t0: p * L + t0 + tsz],
                                 start=(p == 0), stop=(p == 1))
            nc.vector.tensor_copy(rB[:, t0:t0 + tsz], p2[:, :tsz])
        nc.vector.tensor_scalar(out=rB, in0=rB, scalar1=1.0 / C, scalar2=None, op0=MUL)
        nc.vector.memset(epst, 1e-5)
        nc.scalar.activation(rB, rB, AFT.Sqrt, bias=epst)
        nc.vector.reciprocal(rB, rB)
        nc.gpsimd.dma_start(out=stats_dram[1:2, :], in_=rB)
        rbc = big.tile([128, L], F32, tag="du", name="rbc_g")
        nc.gpsimd.dma_start(out=rbc, in_=stats_dram.ap()[1:2, :].partition_broadcast(128))
        uB = [wp.tile([128, L], BF, tag="uin0", name="uB0"),
              wp.tile([64, L], BF, tag="uin1", name="uB1")]
        for p in range(2):
            psz = 128 if p == 0 else 64
            co = p * L
            sl = st[0:psz, co:co + L]
            nc.vector.tensor_tensor(out=sl, in0=sl, in1=rbc[0:psz, :], op=MUL)
            nc.vector.scalar_tensor_tensor(sl, sl, nw[p], res[0:psz, co:co + L], MUL, ADD)
            nc.vector.tensor_scalar(out=sl, in0=sl, scalar1=nb_[p], scalar2=None, op0=ADD)
            nc.vector.tensor_copy(uB[p], sl)

        _emit_stage(nc, pools, Wt, uB, "b", A_vals, partial_b, bc_dram_b, sz_dram_b)

        nc.gpsimd.collective_compute(
            "AllReduce", ADD, replica_groups=groups,
            ins=[partial_b.ap().opt()], outs=[ssum_b.ap().opt()])

        ob = bcp.tile([128, 2 * L], F32, tag="bcB", name="ob_g", bufs=2)
        nc.gpsimd.dma_start(out=ob[:, 0:L], in_=ssum_b.ap()[0:128, :])
        nc.gpsimd.dma_start(out=ob[0:64, L:2 * L], in_=ssum_b.ap()[128:192, :])
        nc.gpsimd.dma_start(out=out_full[0:128, :], in_=ob[:, 0:L])
        nc.gpsimd.dma_start(out=out_full[128:192, :], in_=ob[0:64, L:2 * L])

    nc.compile()
    return nc


_CACHE = {}


def make_in_maps(inputs):
    x = np.asarray(inputs["x"], np.float32)
    in_maps = []
    for core in range(NCORES):
        b, dr = core % 4, core // 4
        xw = x[b].transpose(1, 0, 2).reshape(L, C).T.copy()
        xh_ = x[b].reshape(L, C).T.copy()
        if dr == 1:
            xw = xw[:, ::-1].copy()
            xh_ = xh_[:, ::-1].copy()
        m = {
            "u0_bf": xw.astype(BF16),
            "xres": xh_.astype(np.float32),
            "mask": np.full((128, 1), float(dr), np.float32),
            "maskinv": np.full((128, 1), 1.0 - float(dr), np.float32),
            "normw": np.asarray(inputs["norm_w"], np.float32).reshape(C, 1).copy(),
            "normb": np.asarray(inputs["norm_b"], np.float32).reshape(C, 1).copy(),
            "ident": np.eye(128, dtype=BF16),
        }
        for s, i in (("a", dr), ("b", 2 + dr)):
            m[f"win_{s}"] = np.asarray(inputs["in_proj_w"][i], np.float32).T.copy().astype(BF16)
            m[f"wout_{s}"] = np.asarray(inputs["out_proj_w"][i], np.float32).T.copy().astype(BF16)
            m[f"wxp_{s}"] = np.asarray(inputs["x_proj_w"][i], np.float32).T.copy().astype(BF16)
            m[f"wdt_{s}"] = np.asarray(inputs["dt_proj_w"][i], np.float32).T.copy().astype(BF16)
            cw = np.asarray(inputs["conv_w"][i], np.float32)  # [Di, 3]
            cvd = np.zeros((9 * 128, 128), np.float32)
            for blk in range(NB):
                for k in range(3):
                    r0 = (blk * 3 + k) * 128
                    d = cw[blk * 128:(blk + 1) * 128, k]
                    cvd[r0 + np.arange(128), np.arange(128)] = d
            m[f"convd_{s}"] = cvd.astype(BF16)
            m[f"convb_{s}"] = np.asarray(inputs["conv_b"][i], np.float32).reshape(DI, 1).copy()
            m[f"dtb_{s}"] = np.asarray(inputs["dt_proj_b"][i], np.float32).reshape(DI, 1).copy()
            m[f"dvec_{s}"] = np.asarray(inputs["D"][i], np.float32).reshape(DI, 1).copy()
        in_maps.append(m)
    return in_maps


def get_nc(inputs):
    if "nc" not in _CACHE:
        A_log = np.asarray(inputs["A_log"], np.float32)
        A_vals = (-np.exp(A_log[0, 0, :].astype(np.float64))).astype(np.float32)
        _CACHE["nc"] = build_nc(A_vals)
    return _CACHE["nc"]


def kernel(**inputs):
    nc = get_nc(inputs)
    in_maps = make_in_maps(inputs)
    res = run_bass_kernel_spmd(nc, in_maps, core_ids=list(range(NCORES)))
    out = np.zeros((B, H, W, C), np.float32)
    for b in range(B):
        of = res.results[b]["out_full"]
        out[b] = of.T.reshape(H, W, C)
    return out
